# revision 11
# baseline (speedup 1.0000x reference)
"""GQA attention (B=2,T=2048,D=4096, 32Q/8KV heads, RoPE, causal) on 8 TRN2 cores.

Sharding: core c = (batch b = c//4, head-group g = c%4). Each core computes its
batch's attention for 8 query heads (global 8g..8g+8) + their 2 aligned KV heads
(global 2g..2g+2), and applies its slice of wo -> a partial [T, D] output.
Host sums the 4 head-group partials per batch. No collectives.

Device kernel (per core), bf16 matmuls / f32 accumulation:
  A) QKV projection from host-pre-transposed x^T, RoPE on DVE via pair-swap
     APs, PE-transpose Q,K into [head_dim, tok]; V stays [tok, head_dim].
  B) Per head: scores (K=128 matmuls, 512-wide chunks), additive causal mask
     on the diagonal chunk, fused exp+rowsum on ScalarE, normalize P in bf16,
     PE-transpose P, PV accumulates out^T in PSUM.
  C) Output projection accumulating over the 8 local heads -> f32 partial.
"""
import numpy as np
import ml_dtypes

import concourse.bass as bass
import concourse.mybir as mybir
from concourse import bacc, tile
from concourse.bass_utils import run_bass_kernel_spmd

bf16 = mybir.dt.bfloat16
f32 = mybir.dt.float32
BF = ml_dtypes.bfloat16

B, T, D = 2, 2048, 4096
NQ, NKV, HD = 32, 8, 128
HQ, HKV = 8, 2            # per-core heads
NT = T // 128             # 16 token tiles
NC = D // 128             # 32 contraction chunks
NB = NT // 4              # 4 tq blocks of 512
SCALE = 1.0 / np.sqrt(HD)
NEG = -1e9


def _build_nc():
    nc = bacc.Bacc(None, target_bir_lowering=False)
    xt_ext = nc.declare_dram_parameter("xt", [NT, 128, NC, 128], bf16, isOutput=False)
    wqkv_ext = nc.declare_dram_parameter("wqkv", [128, NC, 1536], bf16, isOutput=False)
    wo_ext = nc.declare_dram_parameter("wo", [128, HQ, D], bf16, isOutput=False)
    rope_ext = nc.declare_dram_parameter("rope", [128, NT, 1024], bf16, isOutput=False)
    mask_ext = nc.declare_dram_parameter("mask", [128, 4, 512], bf16, isOutput=False)
    id_ext = nc.declare_dram_parameter("ident", [128, 128], bf16, isOutput=False)
    out_ext = nc.declare_dram_parameter("out", [T, D], f32, isOutput=True)

    with tile.TileContext(nc) as tc:
        with (
            tc.tile_pool(name="persist", bufs=1) as persist,
        ):
            qt = persist.tile([128, HQ, T], bf16, tag="qt")      # Q^T per head
            kt = persist.tile([128, HKV, T], bf16, tag="kt")     # K^T per head
            vb = persist.tile([128, NT * 256], bf16, tag="vb")   # V [tok, 2*128] per tile
            ident = persist.tile([128, 128], bf16, tag="ident")
            nc.sync.dma_start(ident[:], id_ext[:])
            ones = persist.tile([128, 128], bf16, tag="ones")
            nc.vector.memset(ones[:], 1.0)

            # ---------------- Phase A: projections + rope + transposes -------
            with (
                tc.tile_pool(name="wqkvp", bufs=1) as wqkvp,
                tc.tile_pool(name="xtp", bufs=2) as xtp,
                tc.tile_pool(name="ropep", bufs=2) as ropep,
                tc.tile_pool(name="rotp", bufs=2) as rotp,
                tc.tile_pool(name="psA", bufs=6, space="PSUM") as psA,
                tc.tile_pool(name="ptA", bufs=2, space="PSUM") as ptA,
            ):
                wqkv = wqkvp.tile([128, NC, 1536], bf16, tag="wqkv")
                nc.sync.dma_start(wqkv[:, 0:16, 0:512], wqkv_ext[:, 0:16, 0:512])
                nc.sync.dma_start(wqkv[:, 16:32, 0:512], wqkv_ext[:, 16:32, 0:512])
                nc.gpsimd.dma_start(wqkv[:, :, 512:1024], wqkv_ext[:, :, 512:1024])
                nc.gpsimd.dma_start(wqkv[:, :, 1024:1536], wqkv_ext[:, :, 1024:1536])

                for tau in range(NT):
                    xts = xtp.tile([128, NC, 128], bf16, tag="xt")
                    nc.sync.dma_start(xts[:], xt_ext[tau])
                    rope = ropep.tile([128, 1024], bf16, tag="rope")
                    nc.sync.dma_start(rope[:], rope_ext[:, tau, :])
                    cc, ss = rope[:, 0:512], rope[:, 512:1024]

                    for oc in range(3):
                        ps = psA.tile([128, 512], f32, tag="proj")
                        for c in range(NC):
                            nc.tensor.matmul(
                                ps[:], xts[:, c, :], wqkv[:, c, oc * 512:(oc + 1) * 512],
                                start=(c == 0), stop=(c == NC - 1))
                        rt = rotp.tile([128, 1024], bf16, tag="rot")
                        rot, tmp = rt[:, 0:512], rt[:, 512:1024]
                        if oc < 2:  # 4 q heads
                            _rope(nc, ps[:], cc, ss, rot, tmp)
                            pt = ptA.tile([128, 512], bf16, tag="ptA")
                            for j in range(4):
                                nc.tensor.transpose(
                                    pt[:, j * 128:(j + 1) * 128],
                                    rot[:, j * 128:(j + 1) * 128], ident[:])
                            nc.vector.tensor_copy(
                                qt[:, oc * 4:(oc + 1) * 4, tau * 128:(tau + 1) * 128],
                                pt[:].rearrange("p (h t) -> p h t", h=4))
                        else:  # 2 k heads + 2 v heads
                            _rope(nc, ps[:, 0:256], cc[:, 0:256], ss[:, 0:256],
                                  rot[:, 0:256], tmp[:, 0:256])
                            pt = ptA.tile([128, 512], bf16, tag="ptA")
                            for j in range(2):
                                nc.tensor.transpose(
                                    pt[:, j * 128:(j + 1) * 128],
                                    rot[:, j * 128:(j + 1) * 128], ident[:])
                            nc.vector.tensor_copy(
                                kt[:, :, tau * 128:(tau + 1) * 128],
                                pt[:, 0:256].rearrange("p (h t) -> p h t", h=2))
                            nc.vector.tensor_copy(
                                vb[:, tau * 256:(tau + 1) * 256], ps[:, 256:512])

            # ---------------- Phase B: attention per head --------------------
            with (
                tc.tile_pool(name="wop", bufs=1) as wop,
                tc.tile_pool(name="aotp", bufs=1) as aotp,
            ):
              aot = aotp.tile([128, HQ, T], bf16, tag="aot")
              wo = wop.tile([128, HQ, D], bf16, tag="wo")
              with (
                tc.tile_pool(name="maskp", bufs=1) as maskp,
                tc.tile_pool(name="ptsp", bufs=4) as ptsp,
                tc.tile_pool(name="recp", bufs=2) as recp,
                tc.tile_pool(name="psB", bufs=2, space="PSUM") as psB,
                tc.tile_pool(name="rsB", bufs=2, space="PSUM") as rsB,
                tc.tile_pool(name="otB", bufs=2, space="PSUM") as otB,
              ):
                masks = maskp.tile([128, 4, 512], bf16, tag="mask")
                nc.sync.dma_start(masks[:], mask_ext[:])
                nc.gpsimd.dma_start(wo[:], wo_ext[:])

                for h in range(HQ):
                    kvh = h // 4
                    for b in range(NB):
                        nstrip = 4 * (b + 1)
                        ot = otB.tile([128, 512], f32, tag="ot")
                        rs = rsB.tile([128, 512], f32, tag="rsB")
                        for tp in range(nstrip // 2):
                            ts2 = (2 * tp, 2 * tp + 1)
                            rr = [t - 4 * b for t in ts2]
                            los = [128 * r if r > 0 else 0 for r in rr]
                            plo = los[0]  # pair-common exp range
                            s_ps = psB.tile([128, 2, 512], f32, tag="s")
                            pts = ptsp.tile([128, 2, 512], bf16, tag="pts")
                            for u, t in enumerate(ts2):
                                nc.tensor.matmul(
                                    s_ps[:, u, plo:512],
                                    kt[:, kvh, t * 128:(t + 1) * 128],
                                    qt[:, h, b * 512 + plo:(b + 1) * 512],
                                    start=True, stop=True)
                            for u, t in enumerate(ts2):
                                r = rr[u]
                                if r >= 0:  # triangle tile only
                                    nc.vector.tensor_add(
                                        s_ps[:, u, 128 * r:128 * (r + 1)],
                                        s_ps[:, u, 128 * r:128 * (r + 1)],
                                        masks[:, r, 128 * r:128 * (r + 1)])
                            nc.scalar.activation(
                                pts[:, :, plo:512], s_ps[:, :, plo:512],
                                mybir.ActivationFunctionType.Exp,
                                bias=0.0, scale=SCALE)
                            for u, t in enumerate(ts2):
                                lo = los[u]
                                nc.tensor.matmul(
                                    rs[:, lo:512], ones[:], pts[:, u, lo:512],
                                    start=(t == 0), stop=(t == nstrip - 1))
                                nc.tensor.matmul(
                                    ot[:, lo:512],
                                    vb[:, t * 256 + kvh * 128: t * 256 + (kvh + 1) * 128],
                                    pts[:, u, lo:512],
                                    start=(t == 0), stop=(t == nstrip - 1))
                        recip = recp.tile([128, 512], f32, tag="recip")
                        nc.vector.reciprocal_approx_fast(out=recip[:], in_=rs[:])
                        nc.vector.tensor_mul(
                            aot[:, h, b * 512:(b + 1) * 512], ot[:], recip[:])

              # ---------------- Phase C: output projection -------------------
              with (
                  tc.tile_pool(name="outp", bufs=2) as outp,
                  tc.tile_pool(name="psC", bufs=4, space="PSUM") as psC,
              ):
                  for tau in range(NT):
                      ostage = outp.tile([128, D], f32, tag="ostage")
                      for nck in range(8):
                          o_ps = psC.tile([128, 512], f32, tag="o")
                          for h in range(HQ):
                              nc.tensor.matmul(
                                  o_ps[:], aot[:, h, tau * 128:(tau + 1) * 128],
                                  wo[:, h, nck * 512:(nck + 1) * 512],
                                  start=(h == 0), stop=(h == HQ - 1))
                          nc.vector.tensor_copy(ostage[:, nck * 512:(nck + 1) * 512], o_ps[:])
                          nc.sync.dma_start(
                              out_ext[tau * 128:(tau + 1) * 128, nck * 512:(nck + 1) * 512],
                              ostage[:, nck * 512:(nck + 1) * 512])

    nc.compile()
    return nc


def _rope(nc, ps, cc, ss, rot, tmp):
    """rot = ps*cc + pairswap(ps)*ss   (pairs are consecutive elements)."""
    swap = ps.rearrange("p (i two) -> p i two", two=2)[:, :, ::-1]
    nc.vector.tensor_mul(tmp.rearrange("p (i two) -> p i two", two=2), swap,
                         ss.rearrange("p (i two) -> p i two", two=2))
    nc.vector.tensor_mul(rot, ps, cc)
    nc.vector.tensor_add(rot, rot, tmp)


_NC_CACHE = None


def _get_nc():
    global _NC_CACHE
    if _NC_CACHE is None:
        _NC_CACHE = _build_nc()
    return _NC_CACHE


def _rope_tables():
    i = np.arange(HD // 2, dtype=np.float64)
    theta = np.power(10000.0, -2.0 * i / HD)
    ang = np.outer(np.arange(T, dtype=np.float64), theta)    # [T, 64]
    cos, sin = np.cos(ang), np.sin(ang)
    cc128 = np.repeat(cos, 2, axis=1)                        # [T, 128]
    ss128 = np.stack([-sin, sin], axis=-1).reshape(T, HD)    # [T, 128]
    cc = np.tile(cc128, (1, 4))                              # [T, 512]
    ss = np.tile(ss128, (1, 4))
    ropeccss = np.concatenate([cc, ss], axis=1)              # [T, 1024]
    return np.ascontiguousarray(
        ropeccss.reshape(NT, 128, 1024).transpose(1, 0, 2)).astype(BF)


def _masks():
    # maskT for S^T strips: partition p = tk within strip, free f = tq within
    # block; strip r (0..3) inside the diagonal region. Valid iff tq >= tk.
    p = np.arange(128)[:, None, None]
    r = np.arange(4)[None, :, None]
    f = np.arange(512)[None, None, :]
    return np.where(f >= 128 * r + p, 0.0, NEG).astype(BF)


def _prep_core_inputs(x, wq, wk, wv, wo):
    rope = _rope_tables()
    masks = _masks()
    ident = np.eye(128).astype(BF)
    in_maps = []
    for c in range(8):
        b, g = c // 4, c % 4
        xb = np.asarray(x[b], dtype=np.float32)
        xt = np.ascontiguousarray(
            xb.reshape(NT, 128, NC, 128).transpose(0, 3, 2, 1)).astype(BF)
        wq_g = wq[:, g * 8 * HD:(g + 1) * 8 * HD]
        wk_g = wk[:, g * 2 * HD:(g + 1) * 2 * HD]
        wv_g = wv[:, g * 2 * HD:(g + 1) * 2 * HD]
        W = np.concatenate([wq_g, wk_g, wv_g], axis=1)       # [D, 1536]
        wqkv_t = np.ascontiguousarray(
            W.reshape(NC, 128, 1536).transpose(1, 0, 2)).astype(BF)
        wo_g = wo[g * 8 * HD:(g + 1) * 8 * HD, :]            # [1024, D]
        wo_t = np.ascontiguousarray(
            wo_g.reshape(HQ, 128, D).transpose(1, 0, 2)).astype(BF)
        in_maps.append({
            "xt": xt, "wqkv": wqkv_t, "wo": wo_t,
            "rope": rope, "mask": masks, "ident": ident,
        })
    return in_maps


def _run(inputs, trace=False, trace_kwargs=None):
    x = np.asarray(inputs["x"], dtype=np.float32)
    wq = np.asarray(inputs["wq"], dtype=np.float32)
    wk = np.asarray(inputs["wk"], dtype=np.float32)
    wv = np.asarray(inputs["wv"], dtype=np.float32)
    wo = np.asarray(inputs["wo"], dtype=np.float32)
    nc = _get_nc()
    in_maps = _prep_core_inputs(x, wq, wk, wv, wo)
    res = run_bass_kernel_spmd(nc, in_maps, core_ids=list(range(8)),
                               trace=trace, **(trace_kwargs or {}))
    out = np.zeros((B, T, D), dtype=np.float32)
    for c in range(8):
        out[c // 4] += res.results[c]["out"]
    return out, res


def kernel(**inputs):
    out, _ = _run(inputs)
    return out


# revision 13
# speedup vs baseline: 1.1345x; 1.1345x over previous
"""GQA attention (B=2,T=2048,D=4096, 32Q/8KV heads, RoPE, causal) on 8 TRN2 cores.

Sharding: core c = (batch b = c//4, head-group g = c%4). Each core computes its
batch's attention for 8 query heads (global 8g..8g+8) + their 2 aligned KV heads
(global 2g..2g+2), and applies its slice of wo -> a partial [T, D] output.
Host sums the 4 head-group partials per batch. No collectives.

Device kernel (per core), bf16 matmuls / f32 accumulation & softmax:
  A) QKV projections from host-pre-transposed x^T (PE, 512-wide chunks),
     RoPE on DVE via negative-step pair-swap APs reading the PSUM chunk,
     PE-transpose Q,K into [head_dim, tok] layout; V stays [tok, head_dim].
  B) Per head / 512-token tq block: scores computed TRANSPOSED (S^T strips
     [tk=128, tq=512]; lhsT=K-tile, rhs=Q^T) so exp output P^T feeds the PV
     matmul directly with no P transposes. Causal handling: strips narrowed
     to the valid column range, triangle-tile additive mask on PSUM. Softmax
     denominator via a ones-matrix matmul accumulated across strips (rowsum
     replicated over partitions); normalization folded into the out^T PSUM
     copyback as a multiply with reciprocal_approx_fast. Phase is ScalarE
     (exp)-bound; rowsum matmuls hide under it.
  C) Output projection accumulating over the 8 local heads -> f32 partial.
"""
import numpy as np
import ml_dtypes

import concourse.bass as bass
import concourse.mybir as mybir
from concourse import bacc, tile
from concourse.bass_utils import run_bass_kernel_spmd

bf16 = mybir.dt.bfloat16
f32 = mybir.dt.float32
BF = ml_dtypes.bfloat16

B, T, D = 2, 2048, 4096
NQ, NKV, HD = 32, 8, 128
HQ, HKV = 8, 2            # per-core heads
NT = T // 128             # 16 token tiles
NC = D // 128             # 32 contraction chunks
NB = NT // 4              # 4 tq blocks of 512
SCALE = 1.0 / np.sqrt(HD)
NEG = -1e9


def _build_nc():
    nc = bacc.Bacc(None, target_bir_lowering=False)
    xt_ext = nc.declare_dram_parameter("xt", [NT, 128, NC, 128], bf16, isOutput=False)
    wqkv_ext = nc.declare_dram_parameter("wqkv", [128, NC, 1536], bf16, isOutput=False)
    wo_ext = nc.declare_dram_parameter("wo", [128, HQ, D], bf16, isOutput=False)
    rope_ext = nc.declare_dram_parameter("rope", [128, NT, 1024], bf16, isOutput=False)
    mask_ext = nc.declare_dram_parameter("mask", [128, 4, 512], bf16, isOutput=False)
    id_ext = nc.declare_dram_parameter("ident", [128, 128], bf16, isOutput=False)
    out_ext = nc.declare_dram_parameter("out", [T, D], f32, isOutput=True)

    with tile.TileContext(nc) as tc:
        with (
            tc.tile_pool(name="persist", bufs=1) as persist,
        ):
            qt = persist.tile([128, HQ, T], bf16, tag="qt")      # Q^T per head
            kt = persist.tile([128, HKV, T], bf16, tag="kt")     # K^T per head
            vb = persist.tile([128, NT * 256], bf16, tag="vb")   # V [tok, 2*128] per tile
            ident = persist.tile([128, 128], bf16, tag="ident")
            nc.sync.dma_start(ident[:], id_ext[:])
            ones = persist.tile([128, 128], bf16, tag="ones")
            nc.vector.memset(ones[:], 1.0)

            # ---------------- Phase A: projections + rope + transposes -------
            with (
                tc.tile_pool(name="wqkvp", bufs=1) as wqkvp,
                tc.tile_pool(name="xtp", bufs=2) as xtp,
                tc.tile_pool(name="ropep", bufs=2) as ropep,
                tc.tile_pool(name="rotp", bufs=2) as rotp,
                tc.tile_pool(name="psA", bufs=6, space="PSUM") as psA,
                tc.tile_pool(name="ptA", bufs=2, space="PSUM") as ptA,
            ):
                wqkv = wqkvp.tile([128, NC, 1536], bf16, tag="wqkv")
                nc.sync.dma_start(wqkv[:, 0:16, 0:512], wqkv_ext[:, 0:16, 0:512])
                nc.sync.dma_start(wqkv[:, 16:32, 0:512], wqkv_ext[:, 16:32, 0:512])
                nc.gpsimd.dma_start(wqkv[:, :, 512:1024], wqkv_ext[:, :, 512:1024])
                nc.gpsimd.dma_start(wqkv[:, :, 1024:1536], wqkv_ext[:, :, 1024:1536])

                for tau in range(NT):
                    xts = xtp.tile([128, NC, 128], bf16, tag="xt")
                    nc.sync.dma_start(xts[:], xt_ext[tau])
                    rope = ropep.tile([128, 1024], bf16, tag="rope")
                    nc.sync.dma_start(rope[:], rope_ext[:, tau, :])
                    cc, ss = rope[:, 0:512], rope[:, 512:1024]

                    for oc in range(3):
                        ps = psA.tile([128, 512], f32, tag="proj")
                        for c in range(NC):
                            nc.tensor.matmul(
                                ps[:], xts[:, c, :], wqkv[:, c, oc * 512:(oc + 1) * 512],
                                start=(c == 0), stop=(c == NC - 1))
                        rt = rotp.tile([128, 1024], bf16, tag="rot")
                        rot, tmp = rt[:, 0:512], rt[:, 512:1024]
                        if oc < 2:  # 4 q heads
                            _rope(nc, ps[:], cc, ss, rot, tmp)
                            pt = ptA.tile([128, 512], bf16, tag="ptA")
                            for j in range(4):
                                nc.tensor.transpose(
                                    pt[:, j * 128:(j + 1) * 128],
                                    rot[:, j * 128:(j + 1) * 128], ident[:])
                            nc.vector.tensor_copy(
                                qt[:, oc * 4:(oc + 1) * 4, tau * 128:(tau + 1) * 128],
                                pt[:].rearrange("p (h t) -> p h t", h=4))
                        else:  # 2 k heads + 2 v heads
                            _rope(nc, ps[:, 0:256], cc[:, 0:256], ss[:, 0:256],
                                  rot[:, 0:256], tmp[:, 0:256])
                            pt = ptA.tile([128, 512], bf16, tag="ptA")
                            for j in range(2):
                                nc.tensor.transpose(
                                    pt[:, j * 128:(j + 1) * 128],
                                    rot[:, j * 128:(j + 1) * 128], ident[:])
                            nc.vector.tensor_copy(
                                kt[:, :, tau * 128:(tau + 1) * 128],
                                pt[:, 0:256].rearrange("p (h t) -> p h t", h=2))
                            nc.vector.tensor_copy(
                                vb[:, tau * 256:(tau + 1) * 256], ps[:, 256:512])

            # ---------------- Phase B: attention per head --------------------
            with (
                tc.tile_pool(name="wop", bufs=1) as wop,
                tc.tile_pool(name="aotp", bufs=1) as aotp,
            ):
              aot = aotp.tile([128, HQ, T], bf16, tag="aot")
              wo = wop.tile([128, HQ, D], bf16, tag="wo")
              with (
                tc.tile_pool(name="maskp", bufs=1) as maskp,
                tc.tile_pool(name="ptsp", bufs=4) as ptsp,
                tc.tile_pool(name="recp", bufs=2) as recp,
                tc.tile_pool(name="psB", bufs=4, space="PSUM") as psB,
                tc.tile_pool(name="rsB", bufs=2, space="PSUM") as rsB,
                tc.tile_pool(name="otB", bufs=2, space="PSUM") as otB,
              ):
                masks = maskp.tile([128, 4, 512], bf16, tag="mask")
                nc.sync.dma_start(masks[:], mask_ext[:])
                nc.gpsimd.dma_start(wo[:], wo_ext[:])

                for h in range(HQ):
                    kvh = h // 4
                    for b in range(NB):
                        nstrip = 4 * (b + 1)
                        ot = otB.tile([128, 512], f32, tag="ot")
                        rs = rsB.tile([128, 512], f32, tag="rsB")
                        for t in range(nstrip):
                            # diag strips: only columns f >= 128r are valid
                            r = t - 4 * b
                            lo = 128 * r if r > 0 else 0
                            s_ps = psB.tile([128, 512], f32, tag="s")
                            nc.tensor.matmul(
                                s_ps[:, lo:512], kt[:, kvh, t * 128:(t + 1) * 128],
                                qt[:, h, b * 512 + lo:(b + 1) * 512],
                                start=True, stop=True)
                            if r >= 0:  # triangle tile only
                                nc.vector.tensor_add(
                                    s_ps[:, 128 * r:128 * (r + 1)],
                                    s_ps[:, 128 * r:128 * (r + 1)],
                                    masks[:, r, 128 * r:128 * (r + 1)])
                            pts = ptsp.tile([128, 512], bf16, tag="pts")
                            nc.scalar.activation(
                                pts[:, lo:512], s_ps[:, lo:512],
                                mybir.ActivationFunctionType.Exp,
                                bias=0.0, scale=SCALE)
                            nc.tensor.matmul(
                                rs[:, lo:512], ones[:], pts[:, lo:512],
                                start=(t == 0), stop=(t == nstrip - 1))
                            nc.tensor.matmul(
                                ot[:, lo:512],
                                vb[:, t * 256 + kvh * 128: t * 256 + (kvh + 1) * 128],
                                pts[:, lo:512],
                                start=(t == 0), stop=(t == nstrip - 1))
                        recip = recp.tile([128, 512], f32, tag="recip")
                        nc.vector.reciprocal_approx_fast(out=recip[:], in_=rs[:])
                        nc.vector.tensor_mul(
                            aot[:, h, b * 512:(b + 1) * 512], ot[:], recip[:])

              # ---------------- Phase C: output projection -------------------
              with (
                  tc.tile_pool(name="outp", bufs=2) as outp,
                  tc.tile_pool(name="psC", bufs=4, space="PSUM") as psC,
              ):
                  for tau in range(NT):
                      ostage = outp.tile([128, D], f32, tag="ostage")
                      for nck in range(8):
                          o_ps = psC.tile([128, 512], f32, tag="o")
                          for h in range(HQ):
                              nc.tensor.matmul(
                                  o_ps[:], aot[:, h, tau * 128:(tau + 1) * 128],
                                  wo[:, h, nck * 512:(nck + 1) * 512],
                                  start=(h == 0), stop=(h == HQ - 1))
                          nc.vector.tensor_copy(ostage[:, nck * 512:(nck + 1) * 512], o_ps[:])
                          nc.sync.dma_start(
                              out_ext[tau * 128:(tau + 1) * 128, nck * 512:(nck + 1) * 512],
                              ostage[:, nck * 512:(nck + 1) * 512])

    nc.compile()
    return nc


def _rope(nc, ps, cc, ss, rot, tmp):
    """rot = ps*cc + pairswap(ps)*ss   (pairs are consecutive elements)."""
    swap = ps.rearrange("p (i two) -> p i two", two=2)[:, :, ::-1]
    nc.vector.tensor_mul(tmp.rearrange("p (i two) -> p i two", two=2), swap,
                         ss.rearrange("p (i two) -> p i two", two=2))
    nc.vector.tensor_mul(rot, ps, cc)
    nc.vector.tensor_add(rot, rot, tmp)


_NC_CACHE = None


def _get_nc():
    global _NC_CACHE
    if _NC_CACHE is None:
        _NC_CACHE = _build_nc()
    return _NC_CACHE


def _rope_tables():
    i = np.arange(HD // 2, dtype=np.float64)
    theta = np.power(10000.0, -2.0 * i / HD)
    ang = np.outer(np.arange(T, dtype=np.float64), theta)    # [T, 64]
    cos, sin = np.cos(ang), np.sin(ang)
    cc128 = np.repeat(cos, 2, axis=1)                        # [T, 128]
    ss128 = np.stack([-sin, sin], axis=-1).reshape(T, HD)    # [T, 128]
    cc = np.tile(cc128, (1, 4))                              # [T, 512]
    ss = np.tile(ss128, (1, 4))
    ropeccss = np.concatenate([cc, ss], axis=1)              # [T, 1024]
    return np.ascontiguousarray(
        ropeccss.reshape(NT, 128, 1024).transpose(1, 0, 2)).astype(BF)


def _masks():
    # maskT for S^T strips: partition p = tk within strip, free f = tq within
    # block; strip r (0..3) inside the diagonal region. Valid iff tq >= tk.
    p = np.arange(128)[:, None, None]
    r = np.arange(4)[None, :, None]
    f = np.arange(512)[None, None, :]
    return np.where(f >= 128 * r + p, 0.0, NEG).astype(BF)


def _prep_core_inputs(x, wq, wk, wv, wo):
    rope = _rope_tables()
    masks = _masks()
    ident = np.eye(128).astype(BF)
    in_maps = []
    for c in range(8):
        b, g = c // 4, c % 4
        xb = np.asarray(x[b], dtype=np.float32)
        xt = np.ascontiguousarray(
            xb.reshape(NT, 128, NC, 128).transpose(0, 3, 2, 1)).astype(BF)
        wq_g = wq[:, g * 8 * HD:(g + 1) * 8 * HD]
        wk_g = wk[:, g * 2 * HD:(g + 1) * 2 * HD]
        wv_g = wv[:, g * 2 * HD:(g + 1) * 2 * HD]
        W = np.concatenate([wq_g, wk_g, wv_g], axis=1)       # [D, 1536]
        wqkv_t = np.ascontiguousarray(
            W.reshape(NC, 128, 1536).transpose(1, 0, 2)).astype(BF)
        wo_g = wo[g * 8 * HD:(g + 1) * 8 * HD, :]            # [1024, D]
        wo_t = np.ascontiguousarray(
            wo_g.reshape(HQ, 128, D).transpose(1, 0, 2)).astype(BF)
        in_maps.append({
            "xt": xt, "wqkv": wqkv_t, "wo": wo_t,
            "rope": rope, "mask": masks, "ident": ident,
        })
    return in_maps


def _run(inputs, trace=False, trace_kwargs=None):
    x = np.asarray(inputs["x"], dtype=np.float32)
    wq = np.asarray(inputs["wq"], dtype=np.float32)
    wk = np.asarray(inputs["wk"], dtype=np.float32)
    wv = np.asarray(inputs["wv"], dtype=np.float32)
    wo = np.asarray(inputs["wo"], dtype=np.float32)
    nc = _get_nc()
    in_maps = _prep_core_inputs(x, wq, wk, wv, wo)
    res = run_bass_kernel_spmd(nc, in_maps, core_ids=list(range(8)),
                               trace=trace, **(trace_kwargs or {}))
    out = np.zeros((B, T, D), dtype=np.float32)
    for c in range(8):
        out[c // 4] += res.results[c]["out"]
    return out, res


def kernel(**inputs):
    out, _ = _run(inputs)
    return out


# revision 14
# speedup vs baseline: 1.1612x; 1.0236x over previous
"""GQA attention (B=2,T=2048,D=4096, 32Q/8KV heads, RoPE, causal) on 8 TRN2 cores.

Sharding: core c = (batch b = c//4, head-group g = c%4). Each core computes its
batch's attention for 8 query heads (global 8g..8g+8) + their 2 aligned KV heads
(global 2g..2g+2), and applies its slice of wo -> a partial [T, D] output.
Host sums the 4 head-group partials per batch. No collectives.

Device kernel (per core), bf16 matmuls / f32 accumulation & softmax:
  A) QKV projections from host-pre-transposed x^T (PE, 512-wide chunks),
     RoPE on DVE via negative-step pair-swap APs reading the PSUM chunk,
     PE-transpose Q,K into [head_dim, tok] layout; V stays [tok, head_dim].
  B) Per head / 512-token tq block: scores computed TRANSPOSED (S^T strips
     [tk=128, tq=512]; lhsT=K-tile, rhs=Q^T) so exp output P^T feeds the PV
     matmul directly with no P transposes. Causal handling: strips narrowed
     to the valid column range, triangle-tile additive mask on PSUM. Softmax
     denominator via a ones-matrix matmul accumulated across strips (rowsum
     replicated over partitions); normalization folded into the out^T PSUM
     copyback as a multiply with reciprocal_approx_fast. Phase is ScalarE
     (exp)-bound; rowsum matmuls hide under it.
  C) Output projection accumulating over the 8 local heads -> f32 partial.
"""
import numpy as np
import ml_dtypes

import concourse.bass as bass
import concourse.mybir as mybir
from concourse import bacc, tile
from concourse.bass_utils import run_bass_kernel_spmd

bf16 = mybir.dt.bfloat16
f32 = mybir.dt.float32
BF = ml_dtypes.bfloat16

B, T, D = 2, 2048, 4096
NQ, NKV, HD = 32, 8, 128
HQ, HKV = 8, 2            # per-core heads
NT = T // 128             # 16 token tiles
NC = D // 128             # 32 contraction chunks
NB = NT // 4              # 4 tq blocks of 512
SCALE = 1.0 / np.sqrt(HD)
NEG = -1e9


def _build_nc():
    nc = bacc.Bacc(None, target_bir_lowering=False)
    xt_ext = nc.declare_dram_parameter("xt", [NT, 128, NC, 128], bf16, isOutput=False)
    wqkv_ext = nc.declare_dram_parameter("wqkv", [128, NC, 1536], bf16, isOutput=False)
    wo_ext = nc.declare_dram_parameter("wo", [128, HQ, D], bf16, isOutput=False)
    rope_ext = nc.declare_dram_parameter("rope", [128, NT, 1024], bf16, isOutput=False)
    mask_ext = nc.declare_dram_parameter("mask", [128, 4, 512], bf16, isOutput=False)
    id_ext = nc.declare_dram_parameter("ident", [128, 128], bf16, isOutput=False)
    out_ext = nc.declare_dram_parameter("out", [T, D], f32, isOutput=True)

    with tile.TileContext(nc) as tc:
        with (
            tc.tile_pool(name="persist", bufs=1) as persist,
        ):
            qt = persist.tile([128, HQ, T], bf16, tag="qt")      # Q^T per head
            kt = persist.tile([128, HKV, T], bf16, tag="kt")     # K^T per head
            vb = persist.tile([128, NT * 256], bf16, tag="vb")   # V [tok, 2*128] per tile
            ident = persist.tile([128, 128], bf16, tag="ident")
            nc.sync.dma_start(ident[:], id_ext[:])
            ones = persist.tile([128, 128], bf16, tag="ones")
            nc.vector.memset(ones[:], 1.0)

            # ---------------- Phase A: projections + rope + transposes -------
            with (
                tc.tile_pool(name="wqkvp", bufs=1) as wqkvp,
                tc.tile_pool(name="xtp", bufs=2) as xtp,
                tc.tile_pool(name="ropep", bufs=2) as ropep,
                tc.tile_pool(name="rotp", bufs=2) as rotp,
                tc.tile_pool(name="psA", bufs=6, space="PSUM") as psA,
                tc.tile_pool(name="ptA", bufs=2, space="PSUM") as ptA,
            ):
                wqkv = wqkvp.tile([128, NC, 1536], bf16, tag="wqkv")
                nc.sync.dma_start(wqkv[:, 0:4, :], wqkv_ext[:, 0:4, :])
                for w8 in range(1, 8):
                    nc.gpsimd.dma_start(wqkv[:, w8 * 4:(w8 + 1) * 4, :],
                                        wqkv_ext[:, w8 * 4:(w8 + 1) * 4, :])

                for tau in range(NT):
                    xts = xtp.tile([128, NC, 128], bf16, tag="xt")
                    nc.sync.dma_start(xts[:], xt_ext[tau])
                    rope = ropep.tile([128, 1024], bf16, tag="rope")
                    nc.sync.dma_start(rope[:], rope_ext[:, tau, :])
                    cc, ss = rope[:, 0:512], rope[:, 512:1024]

                    for oc in range(3):
                        ps = psA.tile([128, 512], f32, tag="proj")
                        for c in range(NC):
                            nc.tensor.matmul(
                                ps[:], xts[:, c, :], wqkv[:, c, oc * 512:(oc + 1) * 512],
                                start=(c == 0), stop=(c == NC - 1))
                        rt = rotp.tile([128, 1024], bf16, tag="rot")
                        rot, tmp = rt[:, 0:512], rt[:, 512:1024]
                        if oc < 2:  # 4 q heads
                            _rope(nc, ps[:], cc, ss, rot, tmp)
                            pt = ptA.tile([128, 512], bf16, tag="ptA")
                            for j in range(4):
                                nc.tensor.transpose(
                                    pt[:, j * 128:(j + 1) * 128],
                                    rot[:, j * 128:(j + 1) * 128], ident[:])
                            nc.vector.tensor_copy(
                                qt[:, oc * 4:(oc + 1) * 4, tau * 128:(tau + 1) * 128],
                                pt[:].rearrange("p (h t) -> p h t", h=4))
                        else:  # 2 k heads + 2 v heads
                            _rope(nc, ps[:, 0:256], cc[:, 0:256], ss[:, 0:256],
                                  rot[:, 0:256], tmp[:, 0:256])
                            pt = ptA.tile([128, 512], bf16, tag="ptA")
                            for j in range(2):
                                nc.tensor.transpose(
                                    pt[:, j * 128:(j + 1) * 128],
                                    rot[:, j * 128:(j + 1) * 128], ident[:])
                            nc.vector.tensor_copy(
                                kt[:, :, tau * 128:(tau + 1) * 128],
                                pt[:, 0:256].rearrange("p (h t) -> p h t", h=2))
                            nc.vector.tensor_copy(
                                vb[:, tau * 256:(tau + 1) * 256], ps[:, 256:512])

            # ---------------- Phase B: attention per head --------------------
            with (
                tc.tile_pool(name="wop", bufs=1) as wop,
                tc.tile_pool(name="aotp", bufs=1) as aotp,
            ):
              aot = aotp.tile([128, HQ, T], bf16, tag="aot")
              wo = wop.tile([128, HQ, D], bf16, tag="wo")
              with (
                tc.tile_pool(name="maskp", bufs=1) as maskp,
                tc.tile_pool(name="ptsp", bufs=4) as ptsp,
                tc.tile_pool(name="recp", bufs=2) as recp,
                tc.tile_pool(name="psB", bufs=4, space="PSUM") as psB,
                tc.tile_pool(name="rsB", bufs=2, space="PSUM") as rsB,
                tc.tile_pool(name="otB", bufs=2, space="PSUM") as otB,
              ):
                masks = maskp.tile([128, 4, 512], bf16, tag="mask")
                nc.sync.dma_start(masks[:], mask_ext[:])
                nc.gpsimd.dma_start(wo[:], wo_ext[:])

                for h in range(HQ):
                    kvh = h // 4
                    for b in range(NB):
                        nstrip = 4 * (b + 1)
                        ot = otB.tile([128, 512], f32, tag="ot")
                        rs = rsB.tile([128, 512], f32, tag="rsB")
                        for t in range(nstrip):
                            # diag strips: only columns f >= 128r are valid
                            r = t - 4 * b
                            lo = 128 * r if r > 0 else 0
                            s_ps = psB.tile([128, 512], f32, tag="s")
                            nc.tensor.matmul(
                                s_ps[:, lo:512], kt[:, kvh, t * 128:(t + 1) * 128],
                                qt[:, h, b * 512 + lo:(b + 1) * 512],
                                start=True, stop=True)
                            if r >= 0:  # triangle tile only
                                nc.vector.tensor_add(
                                    s_ps[:, 128 * r:128 * (r + 1)],
                                    s_ps[:, 128 * r:128 * (r + 1)],
                                    masks[:, r, 128 * r:128 * (r + 1)])
                            pts = ptsp.tile([128, 512], bf16, tag="pts")
                            nc.scalar.activation(
                                pts[:, lo:512], s_ps[:, lo:512],
                                mybir.ActivationFunctionType.Exp,
                                bias=0.0, scale=SCALE)
                            nc.tensor.matmul(
                                rs[:, lo:512], ones[:], pts[:, lo:512],
                                start=(t == 0), stop=(t == nstrip - 1))
                            nc.tensor.matmul(
                                ot[:, lo:512],
                                vb[:, t * 256 + kvh * 128: t * 256 + (kvh + 1) * 128],
                                pts[:, lo:512],
                                start=(t == 0), stop=(t == nstrip - 1))
                        recip = recp.tile([128, 512], f32, tag="recip")
                        nc.vector.reciprocal_approx_fast(out=recip[:], in_=rs[:])
                        nc.vector.tensor_mul(
                            aot[:, h, b * 512:(b + 1) * 512], ot[:], recip[:])

              # ---------------- Phase C: output projection -------------------
              with (
                  tc.tile_pool(name="outp", bufs=2) as outp,
                  tc.tile_pool(name="psC", bufs=4, space="PSUM") as psC,
              ):
                  for tau in range(NT):
                      ostage = outp.tile([128, D], f32, tag="ostage")
                      for nck in range(8):
                          o_ps = psC.tile([128, 512], f32, tag="o")
                          for h in range(HQ):
                              nc.tensor.matmul(
                                  o_ps[:], aot[:, h, tau * 128:(tau + 1) * 128],
                                  wo[:, h, nck * 512:(nck + 1) * 512],
                                  start=(h == 0), stop=(h == HQ - 1))
                          nc.vector.tensor_copy(ostage[:, nck * 512:(nck + 1) * 512], o_ps[:])
                          nc.sync.dma_start(
                              out_ext[tau * 128:(tau + 1) * 128, nck * 512:(nck + 1) * 512],
                              ostage[:, nck * 512:(nck + 1) * 512])

    nc.compile()
    return nc


def _rope(nc, ps, cc, ss, rot, tmp):
    """rot = ps*cc + pairswap(ps)*ss   (pairs are consecutive elements)."""
    swap = ps.rearrange("p (i two) -> p i two", two=2)[:, :, ::-1]
    nc.vector.tensor_mul(tmp.rearrange("p (i two) -> p i two", two=2), swap,
                         ss.rearrange("p (i two) -> p i two", two=2))
    nc.vector.tensor_mul(rot, ps, cc)
    nc.vector.tensor_add(rot, rot, tmp)


_NC_CACHE = None


def _get_nc():
    global _NC_CACHE
    if _NC_CACHE is None:
        _NC_CACHE = _build_nc()
    return _NC_CACHE


def _rope_tables():
    i = np.arange(HD // 2, dtype=np.float64)
    theta = np.power(10000.0, -2.0 * i / HD)
    ang = np.outer(np.arange(T, dtype=np.float64), theta)    # [T, 64]
    cos, sin = np.cos(ang), np.sin(ang)
    cc128 = np.repeat(cos, 2, axis=1)                        # [T, 128]
    ss128 = np.stack([-sin, sin], axis=-1).reshape(T, HD)    # [T, 128]
    cc = np.tile(cc128, (1, 4))                              # [T, 512]
    ss = np.tile(ss128, (1, 4))
    ropeccss = np.concatenate([cc, ss], axis=1)              # [T, 1024]
    return np.ascontiguousarray(
        ropeccss.reshape(NT, 128, 1024).transpose(1, 0, 2)).astype(BF)


def _masks():
    # maskT for S^T strips: partition p = tk within strip, free f = tq within
    # block; strip r (0..3) inside the diagonal region. Valid iff tq >= tk.
    p = np.arange(128)[:, None, None]
    r = np.arange(4)[None, :, None]
    f = np.arange(512)[None, None, :]
    return np.where(f >= 128 * r + p, 0.0, NEG).astype(BF)


def _prep_core_inputs(x, wq, wk, wv, wo):
    rope = _rope_tables()
    masks = _masks()
    ident = np.eye(128).astype(BF)
    in_maps = []
    for c in range(8):
        b, g = c // 4, c % 4
        xb = np.asarray(x[b], dtype=np.float32)
        xt = np.ascontiguousarray(
            xb.reshape(NT, 128, NC, 128).transpose(0, 3, 2, 1)).astype(BF)
        wq_g = wq[:, g * 8 * HD:(g + 1) * 8 * HD]
        wk_g = wk[:, g * 2 * HD:(g + 1) * 2 * HD]
        wv_g = wv[:, g * 2 * HD:(g + 1) * 2 * HD]
        W = np.concatenate([wq_g, wk_g, wv_g], axis=1)       # [D, 1536]
        wqkv_t = np.ascontiguousarray(
            W.reshape(NC, 128, 1536).transpose(1, 0, 2)).astype(BF)
        wo_g = wo[g * 8 * HD:(g + 1) * 8 * HD, :]            # [1024, D]
        wo_t = np.ascontiguousarray(
            wo_g.reshape(HQ, 128, D).transpose(1, 0, 2)).astype(BF)
        in_maps.append({
            "xt": xt, "wqkv": wqkv_t, "wo": wo_t,
            "rope": rope, "mask": masks, "ident": ident,
        })
    return in_maps


def _run(inputs, trace=False, trace_kwargs=None):
    x = np.asarray(inputs["x"], dtype=np.float32)
    wq = np.asarray(inputs["wq"], dtype=np.float32)
    wk = np.asarray(inputs["wk"], dtype=np.float32)
    wv = np.asarray(inputs["wv"], dtype=np.float32)
    wo = np.asarray(inputs["wo"], dtype=np.float32)
    nc = _get_nc()
    in_maps = _prep_core_inputs(x, wq, wk, wv, wo)
    res = run_bass_kernel_spmd(nc, in_maps, core_ids=list(range(8)),
                               trace=trace, **(trace_kwargs or {}))
    out = np.zeros((B, T, D), dtype=np.float32)
    for c in range(8):
        out[c // 4] += res.results[c]["out"]
    return out, res


def kernel(**inputs):
    out, _ = _run(inputs)
    return out
